# revision 33
# baseline (speedup 1.0000x reference)
"""Multi-head causal linear attention (B=1, N=2048, D=1024, H=16) on 8 trn2 cores.

Math: reference computes, per head (e=64):
    q = softmax(q_raw, -1) * e**-0.5 ;  k = exp(k_raw)
    out_n = (q_n . KV_n) / (q_n . (kcum_n + EPS)),  KV_n = sum_{j<=n} k_j v_j^T
Because both numerator and denominator are linear in q_n, the softmax
normalization and the e**-0.5 scale cancel exactly; only u = exp(q_raw)
matters.  The EPS term contributes <1e-6 relative and is dropped.  The
v-bias contribution factors out:  out += b_v  (sum_j s_nj / denom ~= 1).

Per-core work (head-parallel, 2 heads/core):
    qvk^T = W_c^T @ x  computed as matmul(lhsT=W_block, rhs=x^T) on PE,
    x^T is pre-transposed on the host so no on-chip transpose is needed.
    Chunked causal linear attention (chunk=128) with the classic
    intra (masked QK^T V) + inter (running KV state) recurrence.

Layout tricks:
  - input DMA is issued first, in k-tile bundles [W_k | x0_k], and the
    first token tile's projection runs k-outer / f-inner so the PE starts
    real (HAM-warming) work as soon as bundle 0 lands.
  - the running KV state is kept block-diagonal [128, 2*65] in bf16 so a
    single matmul (lhsT=UT chunk, K=128 -> FWL) applies BOTH heads' inter
    term, and a single delta matmul (lhsT=ek_tok, rhs=v_aug flat) computes
    both heads' state update (off-diagonal blocks are garbage, never read).
  - per-chunk prepare (transpose/S/mask) is interleaved into the chain so
    the tail stays dense on PE (HAM stays at K=8/8).
"""

import os
from contextlib import ExitStack

import numpy as np

import concourse.bass as bass
import concourse.mybir as mybir
import concourse.tile as tile
from concourse import bacc
from concourse._compat import with_exitstack
from concourse.bass import ts

FP32 = mybir.dt.float32
BF16 = mybir.dt.bfloat16

B, N, D, H = 1, 2048, 1024, 16
E = D // H          # 64 head dim
NCORES = 8
HPC = H // NCORES   # 2 heads per core
F = 3 * HPC * E     # 384 per-core projected features (q | k | v)
KT = D // 128       # 8 contraction tiles
TT = 512            # token tile (projection granularity)
NTT = N // TT       # 4
C = 128             # chunk (tokens) for the causal recurrence
CPT = TT // C       # 4 chunks per token tile
NC = N // C         # 16 chunks total
BK = F + TT         # bundle cols per k-tile: [W(384) | x0(512)]
EA = E + 1          # 65: v columns + ones column

Exp = mybir.ActivationFunctionType.Exp
ActCopy = mybir.ActivationFunctionType.Copy
ActIdent = mybir.ActivationFunctionType.Identity
MULT = mybir.AluOpType.mult
ADD = mybir.AluOpType.add


@with_exitstack
def _emit(ctx: ExitStack, tc, io):
    nc = tc.nc
    bund_d, cf_d, xt1_d, xt23_d, out_d = io

    const = ctx.enter_context(tc.tile_pool(name="const", bufs=1))
    chain = ctx.enter_context(tc.tile_pool(name="chain", bufs=2))
    smtp = ctx.enter_context(tc.tile_pool(name="smtp", bufs=2))
    small = ctx.enter_context(tc.tile_pool(name="small", bufs=3))
    outp = ctx.enter_context(tc.tile_pool(name="outp", bufs=3))
    pproj = ctx.enter_context(tc.tile_pool(name="pproj", bufs=2, space="PSUM"))
    # per-head S^T banks (single-buffered, separate banks so the row-packed
    # matmul pair runs concurrently)
    ps_scr = ctx.enter_context(tc.tile_pool(name="ps_scr", bufs=1, space="PSUM"))
    # one bank, single-buffered: [tr(Ek) 128 | tr(V) 128] bf16
    ps_tr = ctx.enter_context(tc.tile_pool(name="ps_tr", bufs=1, space="PSUM"))
    # one 2KB bank per chunk: [out (130) | dd (130) | unused]; also hosts the
    # warm-up junk and tile-0's third projection accumulator (V)
    ps_od = ctx.enter_context(tc.tile_pool(name="ps_od", bufs=3, space="PSUM"))

    # ---- persistent SBUF ----
    # bund: [ (W_k(384) | x0_k(512)) * 8 | ident 128 ]
    bund_sb = const.tile([128, KT * BK + 128], BF16)
    cf_sb = const.tile([128, 3 + C], FP32)  # [bq|bk|bv|mask]
    xtr_sb = const.tile([128, (NTT - 1) * KT * TT], BF16)  # xt tt=1..3, (tt k t)
    kv_st = [
        const.tile([128, HPC * EA], BF16, name=f"kv{i}") for i in range(2)
    ]  # block-diag state

    id_sb = bund_sb[:, KT * BK : KT * BK + 128]
    bq_sb = cf_sb[:, 0:1]
    bk_sb = cf_sb[:, 1:2]
    bv_sb = cf_sb[:, 2:3]  # per-partition (= per V feature) bias column
    mask_sb = cf_sb[:, 3:]  # [128, 128]  (j, i) 1 iff j<=i

    def w_ap(k, f):
        return bund_sb[:, k * BK + f * 128 : k * BK + (f + 1) * 128]

    def xt_ap(tt, k):
        if tt == 0:
            return bund_sb[:, k * BK + F : k * BK + F + TT]
        base = (tt - 1) * KT * TT + k * TT
        return xtr_sb[:, base : base + TT]

    # ---- input DMA first, all on ONE ring (strictly ordered, full
    # bandwidth each) so early bundles are not starved by the xt stream.
    # Per-k granularity keeps the tile-0 PE drip-fed without long stalls.
    for k in range(KT):
        hi = (k + 1) * BK if k < KT - 1 else KT * BK + 128
        nc.sync.dma_start(bund_sb[:, k * BK : hi], bund_d[:, k * BK : hi])
    nc.sync.dma_start(cf_sb[:, :], cf_d[:, :])
    nc.sync.dma_start(xtr_sb[:, 0 : KT * TT], xt1_d[:, :])
    nc.sync.dma_start(xtr_sb[:, KT * TT : 2 * KT * TT], xt23_d[:, 0 : KT * TT])
    nc.sync.dma_start(xtr_sb[:, 2 * KT * TT :], xt23_d[:, KT * TT :])

    # zero the off-diagonal blocks of both KV state buffers (they are only
    # ever written in their diagonal blocks)
    nc.gpsimd.memset(kv_st[0][:, :], 0.0)
    nc.gpsimd.memset(kv_st[1][:, :], 0.0)
    junk_sb = const.tile([128, 128], BF16, name="junk_sb")
    nc.gpsimd.memset(junk_sb[:, :], 0.0)

    # ---- HAM warm-up: ~2.6us of junk matmuls bridge the DMA-path startup
    # latency so the real tile-0 matmuls run at 2.4 GHz.  They occupy an od
    # bank that the chunk rotation reuses later.
    junk_ps = ps_od.tile([128, 512], FP32, tag="od", name="junkps")
    for _ in range(24):
        nc.tensor.matmul(
            junk_ps[:, 0:128],
            lhsT=junk_sb[:, :],
            rhs=junk_sb[:, :],
            start=True,
            stop=True,
        )

    st = [dict(smt=[None] * CPT, ek=[None] * CPT, va=[None] * CPT) for _ in range(NTT)]
    dma_flip = [0]

    def emit_act(tt, f, pp):
        s = st[tt]
        if f == 0:
            s["UT"] = UT = chain.tile([128, TT], BF16, tag="UT", name=f"UT{tt}")
            nc.scalar.activation(UT[:, :], pp[:, :], Exp, bias=bq_sb[:, 0:1])
        elif f == 1:
            s["EkT"] = EkT = chain.tile([128, TT], BF16, tag="EkT", name=f"EkT{tt}")
            nc.scalar.activation(EkT[:, :], pp[:, :], Exp, bias=bk_sb[:, 0:1])
        else:
            # fold the v-bias into V here: sum_j w_ij (v_j + bv) =
            # num_ij + den_i * bv, so out = num'/den needs no bias add.
            s["VT"] = VT = chain.tile([128, TT], BF16, tag="VT", name=f"VT{tt}")
            nc.scalar.activation(VT[:, :], pp[:, :], ActIdent, bias=bv_sb[:, 0:1])

    def emit_proj_f(tt, f):
        # projection (f-outer): qvk^T[f, t] = sum_d W[d, f] * xT[d, t]
        pp = pproj.tile([128, TT], FP32, tag="proj", name=f"pp{tt}_{f}")
        for k in range(KT):
            nc.tensor.matmul(
                pp[:, :],
                lhsT=w_ap(k, f),
                rhs=xt_ap(tt, k),
                start=(k == 0),
                stop=(k == KT - 1),
            )
        emit_act(tt, f, pp)

    def emit_proj_tile0():
        # k-outer / f-inner: each arriving bundle feeds 3 matmuls, PE ramps
        # with the DMA stream and warms HAM on real work.  The third
        # accumulator (V) borrows an od bank so pproj stays at 2 banks.
        pps = [
            pproj.tile([128, TT], FP32, tag="proj", name="pp0_0"),
            pproj.tile([128, TT], FP32, tag="proj", name="pp0_1"),
            ps_od.tile([128, TT], FP32, tag="od", name="pp0_2"),
        ]
        for k in range(KT):
            for f in range(3):
                nc.tensor.matmul(
                    pps[f][:, :],
                    lhsT=w_ap(k, f),
                    rhs=xt_ap(0, k),
                    start=(k == 0),
                    stop=(k == KT - 1),
                )
        for f in range(3):
            emit_act(0, f, pps[f])

    def emit_prep(tt, cc):
        # per-chunk: token-layout Ek / V (PE transpose + ACT copy), chunk
        # scores S^T and mask
        s = st[tt]
        UT, EkT, VT = s["UT"], s["EkT"], s["VT"]
        # S^T[j, i] = sum_d Ek[j,d] U[i,d]  (row-packed head pair, separate
        # PSUM banks so the two matmuls overlap in the array)
        smts = []
        sps_t = []
        for h in range(HPC):
            sps = ps_scr.tile([128, C], FP32, tag=f"s{h}", name=f"sp{tt}_{cc}_{h}")
            nc.tensor.matmul(
                sps[:, :],
                lhsT=EkT[ts(h, E), ts(cc, C)],
                rhs=UT[ts(h, E), ts(cc, C)],
                start=True,
                stop=True,
                tile_position=(E * h, 0),
            )
            sps_t.append(sps)
        trp = ps_tr.tile([128, 256], BF16, tag="tr", name=f"trp{tt}_{cc}")
        nc.tensor.transpose(trp[:, 0:128], EkT[:, ts(cc, C)], id_sb[:, :])
        nc.tensor.transpose(trp[:, 128:256], VT[:, ts(cc, C)], id_sb[:, :])
        for h in range(HPC):
            sm = smtp.tile([128, C], BF16, tag=f"m{h}", bufs=5, name=f"smt{tt}_{cc}_{h}")
            nc.vector.tensor_mul(sm[:, :], sps_t[h][:, :], mask_sb[:, :])
            smts.append(sm)
        ek_tok = small.tile([128, 128], BF16, tag="ektok", bufs=6, name=f"ek{tt}_{cc}")
        nc.vector.tensor_copy(ek_tok[:, :], trp[:, 0:128])
        v_aug = small.tile(
            [128, HPC, EA], BF16, tag="vaug", bufs=6, name=f"va{tt}_{cc}"
        )
        nc.scalar.copy(
            v_aug[:, :, 0:E],
            trp[:, 128:256].rearrange("p (g e) -> p g e", g=HPC),
        )
        nc.gpsimd.memset(v_aug[:, :, E : E + 1], 1.0)
        s["smt"][cc] = smts
        s["ek"][cc] = ek_tok
        s["va"][cc] = v_aug

    osb_ref = [None]

    def finalize(out_ps, tt, cc):
        c = tt * CPT + cc
        last_chunk = c == NC - 1
        if cc == 0:
            osb_ref[0] = outp.tile([128, CPT, HPC * E], BF16, tag="osb", name=f"o{tt}")
        osb = osb_ref[0]
        o3 = out_ps.rearrange("p (g e) -> p g e", g=HPC)
        rec = small.tile([128, HPC], FP32, tag="rec", name=f"rec{c}")
        nc.vector.reciprocal(rec[:, :], o3[:, :, E])
        for h in range(HPC):
            # v-bias is folded into V, so normalize is a pure per-token scale
            nc.vector.tensor_scalar_mul(
                osb[:, cc, ts(h, E)], o3[:, h, 0:E], rec[:, h : h + 1]
            )
            if last_chunk:
                eng2 = nc.sync if h == 0 else nc.gpsimd
                eng2.dma_start(out_d[ts(c, C), ts(h, E)], osb[:, cc, ts(h, E)])
        if last_chunk:
            return
        eng = nc.gpsimd if dma_flip[0] % 2 else nc.sync
        if tt == NTT - 1:
            dma_flip[0] += 1
            eng.dma_start(out_d[ts(c, C), :], osb[:, cc, :])
        elif cc == CPT - 1:
            dma_flip[0] += 1
            eng.dma_start(
                out_d[ts(tt, TT), :].rearrange("(cc p) f -> p cc f", p=128),
                osb[:, :, :],
            )

    def emit_chain_chunk(tt, cc):
        s = st[tt]
        UT = s["UT"]
        c = tt * CPT + cc
        smts, ek_tok, v_aug = s["smt"][cc], s["ek"][cc], s["va"][cc]
        vflat = v_aug.rearrange("p g e -> p (g e)")
        od = ps_od.tile([128, 512], FP32, tag="od", name=f"od{c}")
        out_ps = od[:, 0 : HPC * EA]
        # NOTE on start=: start=True clears has_written for the WHOLE bank,
        # so only the FIRST matmul touching this od bank per chunk may set
        # it; later matmuls rely on per-element overwrite-where-unwritten.
        if c < NC - 1:
            # both heads' state delta in one matmul; off-diagonal blocks of
            # dd are cross-head garbage and never read.  Emitted first so
            # the DVE state update has slack before the next chunk's inter.
            dd = od[:, HPC * EA : 2 * HPC * EA]
            nc.tensor.matmul(
                dd[:, :],
                lhsT=ek_tok[:, :],
                rhs=vflat[:, :],
                start=True,
                stop=True,
            )
            nxt = kv_st[c % 2]
            if c == 0:
                nc.vector.tensor_copy(nxt[0:E, 0:EA], dd[0:E, 0:EA])
                nc.vector.tensor_copy(nxt[E:128, EA:], dd[E:128, EA:])
            else:
                prv = kv_st[(c - 1) % 2]
                nc.vector.tensor_add(nxt[0:E, 0:EA], dd[0:E, 0:EA], prv[0:E, 0:EA])
                nc.vector.tensor_add(nxt[E:128, EA:], dd[E:128, EA:], prv[E:128, EA:])
        for h in range(HPC):
            nc.tensor.matmul(
                out_ps[:, h * EA : (h + 1) * EA],
                lhsT=smts[h][:, :],
                rhs=v_aug[:, h, :],
                start=(c == NC - 1 and h == 0),
                stop=(c == 0 and h == HPC - 1),
            )
        if c > 0:
            # both heads' inter term in one K=128 matmul against the
            # block-diagonal bf16 state
            nc.tensor.matmul(
                out_ps[:, :],
                lhsT=UT[:, ts(cc, C)],
                rhs=kv_st[(c - 1) % 2][:, :],
                start=False,
                stop=True,
            )
        finalize(out_ps, tt, cc)

    def emit_filler(n):
        # tiny junk matmuls into an idle S bank keep the PE activity monitor
        # from re-throttling during the dependency-bound tail
        jp = ps_scr.tile([128, C], FP32, tag="s0", name="fill")
        for _ in range(n):
            nc.tensor.matmul(
                jp[0:64, 0:64],
                lhsT=junk_sb[:, 0:64],
                rhs=junk_sb[:, 0:64],
                start=True,
                stop=True,
            )

    # ---- emission: tile-0 projection (k-outer); tile-1's f0 projection goes
    # right behind it so the PE has work while the scalar engine runs the
    # tile-0 exps.  Then per tile: chain chunks with next-chunk prepare and
    # next-tile projection woven in between.  The last tile's prepares are
    # pulled ahead into tile 2 so its chain runs back-to-back on PE.
    emit_proj_tile0()
    emit_proj_f(1, 0)
    emit_prep(0, 0)
    for tt in range(NTT):
        if tt == 0:
            slices = [lambda: emit_proj_f(1, 1), lambda: emit_proj_f(1, 2), None]
        elif tt == 1:
            slices = [lambda f=f: emit_proj_f(2, f) for f in range(3)]
        elif tt == 2:
            # front-load tile-3's projection so its prepares can spread
            slices = [
                lambda: (emit_proj_f(3, 0), emit_proj_f(3, 1)),
                lambda: emit_proj_f(3, 2),
                lambda: (emit_prep(3, 0), emit_prep(3, 1)),
            ]
        else:
            slices = [None] * 3
        for cc in range(CPT):
            emit_chain_chunk(tt, cc)
            if cc < CPT - 1:
                if tt < NTT - 1:
                    emit_prep(tt, cc + 1)
                    if slices[cc] is not None:
                        slices[cc]()
                else:
                    emit_filler(6)
        if tt < NTT - 2:
            emit_prep(tt + 1, 0)
        elif tt == NTT - 2:
            emit_prep(NTT - 1, 2)
            emit_prep(NTT - 1, 3)


def build_nc():
    nc = bacc.Bacc(
        "TRN2",
        target_bir_lowering=False,
        debug=False,
        enable_asserts=False,
        num_devices=NCORES,
    )
    bund_d = nc.dram_tensor(
        "bund", [128, KT * BK + 128], BF16, kind="ExternalInput"
    ).ap()
    cf_d = nc.dram_tensor("cf", [128, 3 + C], FP32, kind="ExternalInput").ap()
    xt1_d = nc.dram_tensor("xt1", [128, KT * TT], BF16, kind="ExternalInput").ap()
    xt23_d = nc.dram_tensor(
        "xt23", [128, 2 * KT * TT], BF16, kind="ExternalInput"
    ).ap()
    out_d = nc.dram_tensor("out", [N, HPC * E], BF16, kind="ExternalOutput").ap()
    io = (bund_d, cf_d, xt1_d, xt23_d, out_d)
    with tile.TileContext(nc) as tc:
        _emit(tc, io)
    nc.compile()
    return nc


def host_inputs(x, W_qvk, b_qvk):
    """Full inputs -> per-core in_maps (host-side shard + transpose)."""
    import ml_dtypes

    x = np.asarray(x, dtype=np.float32).reshape(N, D)
    W = np.asarray(W_qvk, dtype=np.float32)
    b = np.asarray(b_qvk, dtype=np.float32)
    xt = x.T.astype(ml_dtypes.bfloat16)  # (D, N)

    def pack(a):  # (D, M) -> (128, KT*M), partition-contiguous
        kt, m = a.shape[0] // 128, a.shape[1]
        return np.ascontiguousarray(
            a.reshape(kt, 128, m).transpose(1, 0, 2).reshape(128, kt * m)
        )

    xtp = [pack(xt[:, tt * TT : (tt + 1) * TT]) for tt in range(NTT)]
    xt1 = xtp[1]
    xt23 = np.ascontiguousarray(np.concatenate([xtp[2], xtp[3]], axis=1))
    ident = np.eye(128, dtype=ml_dtypes.bfloat16)

    tri = np.tril(np.ones((C, C), dtype=np.float32))  # [i, j] valid j<=i
    mask = np.ascontiguousarray(tri.T)  # [j, i] 1 iff j<=i

    in_maps = []
    for core in range(NCORES):
        heads = [HPC * core + i for i in range(HPC)]
        # torch.chunk order in reference: q, v, k
        qcols = np.concatenate([np.arange(E * h, E * h + E) for h in heads])
        vcols = qcols + D
        kcols = qcols + 2 * D
        Wc = pack(
            np.concatenate([W[:, qcols], W[:, kcols], W[:, vcols]], axis=1).astype(
                ml_dtypes.bfloat16
            )
        )
        # bundle layout per k: [W_k (384) | x0_k (512)], then ident
        bund = np.empty((128, KT * BK + 128), dtype=ml_dtypes.bfloat16)
        for k in range(KT):
            bund[:, k * BK : k * BK + F] = Wc[:, k * F : (k + 1) * F]
            bund[:, k * BK + F : (k + 1) * BK] = xtp[0][:, k * TT : (k + 1) * TT]
        bund[:, KT * BK :] = ident
        bq = b[qcols].reshape(128, 1)
        bk = b[kcols].reshape(128, 1)
        bv = b[vcols].reshape(128, 1)
        cf = np.ascontiguousarray(
            np.concatenate([bq, bk, bv, mask], axis=1, dtype=np.float32)
        )
        in_maps.append(dict(bund=bund, cf=cf, xt1=xt1, xt23=xt23))
    return in_maps


_CACHE = {}


def kernel(x, W_qvk, b_qvk, head_num):
    assert int(np.asarray(head_num)) == H
    if "nc" not in _CACHE:
        _CACHE["nc"] = build_nc()
    nc = _CACHE["nc"]
    in_maps = host_inputs(x, W_qvk, b_qvk)
    from concourse.bass_utils import run_bass_kernel_spmd

    res = run_bass_kernel_spmd(
        nc,
        in_maps,
        core_ids=list(range(NCORES)),
        trace=bool(int(os.environ.get("KERNEL_TRACE", "0"))),
    )
    _CACHE["last_result"] = res
    out = np.concatenate(
        [np.asarray(r["out"], dtype=np.float32) for r in res.results], axis=1
    )
    return out.reshape(B, N, D)


# revision 35
# speedup vs baseline: 1.0618x; 1.0618x over previous
"""Multi-head causal linear attention (B=1, N=2048, D=1024, H=16) on 8 trn2 cores.

Math: reference computes, per head (e=64):
    q = softmax(q_raw, -1) * e**-0.5 ;  k = exp(k_raw)
    out_n = (q_n . KV_n) / (q_n . (kcum_n + EPS)),  KV_n = sum_{j<=n} k_j v_j^T
Because both numerator and denominator are linear in q_n, the softmax
normalization and the e**-0.5 scale cancel exactly; only u = exp(q_raw)
matters.  The EPS term contributes <1e-6 relative and is dropped.  The
v-bias contribution factors out:  out += b_v  (sum_j s_nj / denom ~= 1).

Per-core work (head-parallel, 2 heads/core):
    qvk^T = W_c^T @ x  computed as matmul(lhsT=W_block, rhs=x^T) on PE,
    x^T is pre-transposed on the host so no on-chip transpose is needed.
    Chunked causal linear attention (chunk=128) with the classic
    intra (masked QK^T V) + inter (running KV state) recurrence.

Layout tricks:
  - input DMA is issued first, in k-tile bundles [W_k | x0_k], and the
    first token tile's projection runs k-outer / f-inner so the PE starts
    real (HAM-warming) work as soon as bundle 0 lands.
  - the running KV state is kept block-diagonal [128, 2*65] in bf16 so a
    single matmul (lhsT=UT chunk, K=128 -> FWL) applies BOTH heads' inter
    term, and a single delta matmul (lhsT=ek_tok, rhs=v_aug flat) computes
    both heads' state update (off-diagonal blocks are garbage, never read).
  - per-chunk prepare (transpose/S/mask) is interleaved into the chain so
    the tail stays dense on PE (HAM stays at K=8/8).
"""

import os
from contextlib import ExitStack

import numpy as np

import concourse.bass as bass
import concourse.mybir as mybir
import concourse.tile as tile
from concourse import bacc
from concourse._compat import with_exitstack
from concourse.bass import ts

FP32 = mybir.dt.float32
BF16 = mybir.dt.bfloat16

B, N, D, H = 1, 2048, 1024, 16
E = D // H          # 64 head dim
NCORES = 8
HPC = H // NCORES   # 2 heads per core
F = 3 * HPC * E     # 384 per-core projected features (q | k | v)
KT = D // 128       # 8 contraction tiles
TT = 512            # token tile (projection granularity)
NTT = N // TT       # 4
C = 128             # chunk (tokens) for the causal recurrence
CPT = TT // C       # 4 chunks per token tile
NC = N // C         # 16 chunks total
BK = F + TT         # bundle cols per k-tile: [W(384) | x0(512)]
EA = E + 1          # 65: v columns + ones column

Exp = mybir.ActivationFunctionType.Exp
ActCopy = mybir.ActivationFunctionType.Copy
ActIdent = mybir.ActivationFunctionType.Identity
MULT = mybir.AluOpType.mult
ADD = mybir.AluOpType.add


@with_exitstack
def _emit(ctx: ExitStack, tc, io):
    nc = tc.nc
    bund_d, cf_d, xt1_d, xt23_d, out_d = io

    const = ctx.enter_context(tc.tile_pool(name="const", bufs=1))
    chain = ctx.enter_context(tc.tile_pool(name="chain", bufs=2))
    smtp = ctx.enter_context(tc.tile_pool(name="smtp", bufs=2))
    small = ctx.enter_context(tc.tile_pool(name="small", bufs=3))
    outp = ctx.enter_context(tc.tile_pool(name="outp", bufs=3))
    pproj = ctx.enter_context(tc.tile_pool(name="pproj", bufs=2, space="PSUM"))
    # per-head S^T banks (single-buffered, separate banks so the row-packed
    # matmul pair runs concurrently)
    ps_scr = ctx.enter_context(tc.tile_pool(name="ps_scr", bufs=1, space="PSUM"))
    # one bank, single-buffered: [tr(Ek) 128 | tr(V) 128] bf16
    ps_tr = ctx.enter_context(tc.tile_pool(name="ps_tr", bufs=1, space="PSUM"))
    # one 2KB bank per chunk: [out (130) | dd (130) | unused]; also hosts the
    # warm-up junk and tile-0's third projection accumulator (V)
    ps_od = ctx.enter_context(tc.tile_pool(name="ps_od", bufs=3, space="PSUM"))

    # ---- persistent SBUF ----
    # bund: [ (W_k(384) | x0_k(512)) * 8 | ident 128 ]
    bund_sb = const.tile([128, KT * BK + 128], BF16)
    cf_sb = const.tile([128, 3 + C], FP32)  # [bq|bk|bv|mask]
    xtr_sb = const.tile([128, (NTT - 1) * KT * TT], BF16)  # xt tt=1..3, (tt k t)
    kv_st = [
        const.tile([128, HPC * EA], BF16, name=f"kv{i}") for i in range(2)
    ]  # block-diag state

    id_sb = bund_sb[:, KT * BK : KT * BK + 128]
    bq_sb = cf_sb[:, 0:1]
    bk_sb = cf_sb[:, 1:2]
    bv_sb = cf_sb[:, 2:3]  # per-partition (= per V feature) bias column
    mask_sb = cf_sb[:, 3:]  # [128, 128]  (j, i) 1 iff j<=i

    def w_ap(k, f):
        return bund_sb[:, k * BK + f * 128 : k * BK + (f + 1) * 128]

    def xt_ap(tt, k):
        if tt == 0:
            return bund_sb[:, k * BK + F : k * BK + F + TT]
        base = (tt - 1) * KT * TT + k * TT
        return xtr_sb[:, base : base + TT]

    # ---- input DMA first, all on ONE ring (strictly ordered, full
    # bandwidth each) so early bundles are not starved by the xt stream.
    # Per-k granularity keeps the tile-0 PE drip-fed without long stalls.
    for k in range(KT):
        hi = (k + 1) * BK if k < KT - 1 else KT * BK + 128
        nc.sync.dma_start(bund_sb[:, k * BK : hi], bund_d[:, k * BK : hi])
    nc.sync.dma_start(cf_sb[:, :], cf_d[:, :])
    nc.sync.dma_start(xtr_sb[:, 0 : KT * TT], xt1_d[:, :])
    nc.sync.dma_start(xtr_sb[:, KT * TT : 2 * KT * TT], xt23_d[:, 0 : KT * TT])
    nc.sync.dma_start(xtr_sb[:, 2 * KT * TT :], xt23_d[:, KT * TT :])

    # zero the off-diagonal blocks of both KV state buffers (they are only
    # ever written in their diagonal blocks)
    nc.gpsimd.memset(kv_st[0][:, :], 0.0)
    nc.gpsimd.memset(kv_st[1][:, :], 0.0)
    junk_sb = const.tile([128, 128], BF16, name="junk_sb")
    nc.gpsimd.memset(junk_sb[:, :], 0.0)

    # ---- HAM warm-up: ~2.6us of junk matmuls bridge the DMA-path startup
    # latency so the real tile-0 matmuls run at 2.4 GHz.  They occupy an od
    # bank that the chunk rotation reuses later.
    junk_ps = ps_od.tile([128, 512], FP32, tag="od", name="junkps")
    for _ in range(24):
        nc.tensor.matmul(
            junk_ps[:, 0:128],
            lhsT=junk_sb[:, :],
            rhs=junk_sb[:, :],
            start=True,
            stop=True,
        )

    st = [dict(smt=[None] * CPT, ek=[None] * CPT, va=[None] * CPT) for _ in range(NTT)]
    dma_flip = [0]

    def emit_act(tt, f, pp):
        s = st[tt]
        if f == 0:
            s["UT"] = UT = chain.tile([128, TT], BF16, tag="UT", name=f"UT{tt}")
            nc.scalar.activation(UT[:, :], pp[:, :], Exp, bias=bq_sb[:, 0:1])
        elif f == 1:
            s["EkT"] = EkT = chain.tile([128, TT], BF16, tag="EkT", name=f"EkT{tt}")
            nc.scalar.activation(EkT[:, :], pp[:, :], Exp, bias=bk_sb[:, 0:1])
        else:
            # fold the v-bias into V here: sum_j w_ij (v_j + bv) =
            # num_ij + den_i * bv, so out = num'/den needs no bias add.
            s["VT"] = VT = chain.tile([128, TT], BF16, tag="VT", name=f"VT{tt}")
            nc.scalar.activation(VT[:, :], pp[:, :], ActIdent, bias=bv_sb[:, 0:1])

    def emit_proj_f(tt, f):
        # projection (f-outer): qvk^T[f, t] = sum_d W[d, f] * xT[d, t]
        pp = pproj.tile([128, TT], FP32, tag="proj", name=f"pp{tt}_{f}")
        for k in range(KT):
            nc.tensor.matmul(
                pp[:, :],
                lhsT=w_ap(k, f),
                rhs=xt_ap(tt, k),
                start=(k == 0),
                stop=(k == KT - 1),
            )
        emit_act(tt, f, pp)

    def emit_proj_tile0():
        # k-outer / f-inner: each arriving bundle feeds 3 matmuls, PE ramps
        # with the DMA stream and warms HAM on real work.  The third
        # accumulator (V) borrows an od bank so pproj stays at 2 banks.
        pps = [
            pproj.tile([128, TT], FP32, tag="proj", name="pp0_0"),
            pproj.tile([128, TT], FP32, tag="proj", name="pp0_1"),
            ps_od.tile([128, TT], FP32, tag="od", name="pp0_2"),
        ]
        for k in range(KT):
            for f in range(3):
                nc.tensor.matmul(
                    pps[f][:, :],
                    lhsT=w_ap(k, f),
                    rhs=xt_ap(0, k),
                    start=(k == 0),
                    stop=(k == KT - 1),
                )
        for f in range(3):
            emit_act(0, f, pps[f])

    def emit_prep(tt, cc):
        # per-chunk: token-layout Ek / V (PE transpose + ACT copy), chunk
        # scores S^T and mask
        s = st[tt]
        UT, EkT, VT = s["UT"], s["EkT"], s["VT"]
        # S^T[j, i] = sum_d Ek[j,d] U[i,d]  (row-packed head pair, separate
        # PSUM banks so the two matmuls overlap in the array)
        smts = []
        sps_t = []
        for h in range(HPC):
            sps = ps_scr.tile([128, C], FP32, tag=f"s{h}", name=f"sp{tt}_{cc}_{h}")
            nc.tensor.matmul(
                sps[:, :],
                lhsT=EkT[ts(h, E), ts(cc, C)],
                rhs=UT[ts(h, E), ts(cc, C)],
                start=True,
                stop=True,
                tile_position=(E * h, 0),
            )
            sps_t.append(sps)
        trp = ps_tr.tile([128, 256], BF16, tag="tr", name=f"trp{tt}_{cc}")
        nc.tensor.transpose(trp[:, 0:128], EkT[:, ts(cc, C)], id_sb[:, :])
        nc.tensor.transpose(trp[:, 128:256], VT[:, ts(cc, C)], id_sb[:, :])
        for h in range(HPC):
            sm = smtp.tile([128, C], BF16, tag=f"m{h}", bufs=5, name=f"smt{tt}_{cc}_{h}")
            nc.vector.tensor_mul(sm[:, :], sps_t[h][:, :], mask_sb[:, :])
            smts.append(sm)
        ek_tok = small.tile([128, 128], BF16, tag="ektok", bufs=6, name=f"ek{tt}_{cc}")
        nc.scalar.copy(ek_tok[:, :], trp[:, 0:128])
        v_aug = small.tile(
            [128, HPC, EA], BF16, tag="vaug", bufs=6, name=f"va{tt}_{cc}"
        )
        nc.scalar.copy(
            v_aug[:, :, 0:E],
            trp[:, 128:256].rearrange("p (g e) -> p g e", g=HPC),
        )
        nc.gpsimd.memset(v_aug[:, :, E : E + 1], 1.0)
        s["smt"][cc] = smts
        s["ek"][cc] = ek_tok
        s["va"][cc] = v_aug

    osb_ref = [None]

    def finalize(out_ps, tt, cc):
        c = tt * CPT + cc
        last_chunk = c == NC - 1
        if cc == 0:
            osb_ref[0] = outp.tile([128, CPT, HPC * E], BF16, tag="osb", name=f"o{tt}")
        osb = osb_ref[0]
        o3 = out_ps.rearrange("p (g e) -> p g e", g=HPC)
        rec = small.tile([128, HPC], FP32, tag="rec", name=f"rec{c}")
        nc.vector.reciprocal(rec[:, :], o3[:, :, E])
        for h in range(HPC):
            # v-bias is folded into V, so normalize is a pure per-token scale
            nc.vector.tensor_scalar_mul(
                osb[:, cc, ts(h, E)], o3[:, h, 0:E], rec[:, h : h + 1]
            )
            if last_chunk:
                eng2 = nc.sync if h == 0 else nc.gpsimd
                eng2.dma_start(out_d[ts(c, C), ts(h, E)], osb[:, cc, ts(h, E)])
        if last_chunk:
            return
        eng = nc.gpsimd if dma_flip[0] % 2 else nc.sync
        if tt == NTT - 1:
            dma_flip[0] += 1
            eng.dma_start(out_d[ts(c, C), :], osb[:, cc, :])
        elif cc == CPT - 1:
            dma_flip[0] += 1
            eng.dma_start(
                out_d[ts(tt, TT), :].rearrange("(cc p) f -> p cc f", p=128),
                osb[:, :, :],
            )

    def emit_chain_chunk(tt, cc):
        s = st[tt]
        UT = s["UT"]
        c = tt * CPT + cc
        smts, ek_tok, v_aug = s["smt"][cc], s["ek"][cc], s["va"][cc]
        vflat = v_aug.rearrange("p g e -> p (g e)")
        od = ps_od.tile([128, 512], FP32, tag="od", name=f"od{c}")
        out_ps = od[:, 0 : HPC * EA]
        # NOTE on start=: start=True clears has_written for the WHOLE bank,
        # so only the FIRST matmul touching this od bank per chunk may set
        # it; later matmuls rely on per-element overwrite-where-unwritten.
        if c < NC - 1:
            # both heads' state delta in one matmul; off-diagonal blocks of
            # dd are cross-head garbage and never read.  Emitted first so
            # the DVE state update has slack before the next chunk's inter.
            dd = od[:, HPC * EA : 2 * HPC * EA]
            nc.tensor.matmul(
                dd[:, :],
                lhsT=ek_tok[:, :],
                rhs=vflat[:, :],
                start=True,
                stop=True,
            )
            nxt = kv_st[c % 2]
            if c == 0:
                nc.vector.tensor_copy(nxt[0:E, 0:EA], dd[0:E, 0:EA])
                nc.vector.tensor_copy(nxt[E:128, EA:], dd[E:128, EA:])
            else:
                prv = kv_st[(c - 1) % 2]
                nc.vector.tensor_add(nxt[0:E, 0:EA], dd[0:E, 0:EA], prv[0:E, 0:EA])
                nc.vector.tensor_add(nxt[E:128, EA:], dd[E:128, EA:], prv[E:128, EA:])
        for h in range(HPC):
            nc.tensor.matmul(
                out_ps[:, h * EA : (h + 1) * EA],
                lhsT=smts[h][:, :],
                rhs=v_aug[:, h, :],
                start=(c == NC - 1 and h == 0),
                stop=(c == 0 and h == HPC - 1),
            )
        if c > 0:
            # both heads' inter term in one K=128 matmul against the
            # block-diagonal bf16 state
            nc.tensor.matmul(
                out_ps[:, :],
                lhsT=UT[:, ts(cc, C)],
                rhs=kv_st[(c - 1) % 2][:, :],
                start=False,
                stop=True,
            )
        finalize(out_ps, tt, cc)

    def emit_filler(n):
        # tiny junk matmuls into an idle S bank keep the PE activity monitor
        # from re-throttling during the dependency-bound tail
        jp = ps_scr.tile([128, C], FP32, tag="s0", name="fill")
        for _ in range(n):
            nc.tensor.matmul(
                jp[0:64, 0:64],
                lhsT=junk_sb[:, 0:64],
                rhs=junk_sb[:, 0:64],
                start=True,
                stop=True,
            )

    # ---- emission: tile-0 projection (k-outer); tile-1's f0 projection goes
    # right behind it so the PE has work while the scalar engine runs the
    # tile-0 exps.  Then per tile: chain chunks with next-chunk prepare and
    # next-tile projection woven in between.
    emit_proj_tile0()
    emit_proj_f(1, 0)
    emit_prep(0, 0)
    for tt in range(NTT):
        if tt == 0:
            slices = [lambda: emit_proj_f(1, 1), lambda: emit_proj_f(1, 2), None]
        elif tt < NTT - 1:
            slices = [lambda f=f: emit_proj_f(tt + 1, f) for f in range(3)]
        else:
            slices = [None] * 3
        for cc in range(CPT):
            emit_chain_chunk(tt, cc)
            if cc < CPT - 1:
                emit_prep(tt, cc + 1)
                if slices[cc] is not None:
                    slices[cc]()
        if tt < NTT - 1:
            emit_prep(tt + 1, 0)


def build_nc():
    nc = bacc.Bacc(
        "TRN2",
        target_bir_lowering=False,
        debug=False,
        enable_asserts=False,
        num_devices=NCORES,
    )
    bund_d = nc.dram_tensor(
        "bund", [128, KT * BK + 128], BF16, kind="ExternalInput"
    ).ap()
    cf_d = nc.dram_tensor("cf", [128, 3 + C], FP32, kind="ExternalInput").ap()
    xt1_d = nc.dram_tensor("xt1", [128, KT * TT], BF16, kind="ExternalInput").ap()
    xt23_d = nc.dram_tensor(
        "xt23", [128, 2 * KT * TT], BF16, kind="ExternalInput"
    ).ap()
    out_d = nc.dram_tensor("out", [N, HPC * E], BF16, kind="ExternalOutput").ap()
    io = (bund_d, cf_d, xt1_d, xt23_d, out_d)
    with tile.TileContext(nc) as tc:
        _emit(tc, io)
    nc.compile()
    return nc


def host_inputs(x, W_qvk, b_qvk):
    """Full inputs -> per-core in_maps (host-side shard + transpose)."""
    import ml_dtypes

    x = np.asarray(x, dtype=np.float32).reshape(N, D)
    W = np.asarray(W_qvk, dtype=np.float32)
    b = np.asarray(b_qvk, dtype=np.float32)
    xt = x.T.astype(ml_dtypes.bfloat16)  # (D, N)

    def pack(a):  # (D, M) -> (128, KT*M), partition-contiguous
        kt, m = a.shape[0] // 128, a.shape[1]
        return np.ascontiguousarray(
            a.reshape(kt, 128, m).transpose(1, 0, 2).reshape(128, kt * m)
        )

    xtp = [pack(xt[:, tt * TT : (tt + 1) * TT]) for tt in range(NTT)]
    xt1 = xtp[1]
    xt23 = np.ascontiguousarray(np.concatenate([xtp[2], xtp[3]], axis=1))
    ident = np.eye(128, dtype=ml_dtypes.bfloat16)

    tri = np.tril(np.ones((C, C), dtype=np.float32))  # [i, j] valid j<=i
    mask = np.ascontiguousarray(tri.T)  # [j, i] 1 iff j<=i

    in_maps = []
    for core in range(NCORES):
        heads = [HPC * core + i for i in range(HPC)]
        # torch.chunk order in reference: q, v, k
        qcols = np.concatenate([np.arange(E * h, E * h + E) for h in heads])
        vcols = qcols + D
        kcols = qcols + 2 * D
        Wc = pack(
            np.concatenate([W[:, qcols], W[:, kcols], W[:, vcols]], axis=1).astype(
                ml_dtypes.bfloat16
            )
        )
        # bundle layout per k: [W_k (384) | x0_k (512)], then ident
        bund = np.empty((128, KT * BK + 128), dtype=ml_dtypes.bfloat16)
        for k in range(KT):
            bund[:, k * BK : k * BK + F] = Wc[:, k * F : (k + 1) * F]
            bund[:, k * BK + F : (k + 1) * BK] = xtp[0][:, k * TT : (k + 1) * TT]
        bund[:, KT * BK :] = ident
        bq = b[qcols].reshape(128, 1)
        bk = b[kcols].reshape(128, 1)
        bv = b[vcols].reshape(128, 1)
        cf = np.ascontiguousarray(
            np.concatenate([bq, bk, bv, mask], axis=1, dtype=np.float32)
        )
        in_maps.append(dict(bund=bund, cf=cf, xt1=xt1, xt23=xt23))
    return in_maps


_CACHE = {}


def kernel(x, W_qvk, b_qvk, head_num):
    assert int(np.asarray(head_num)) == H
    if "nc" not in _CACHE:
        _CACHE["nc"] = build_nc()
    nc = _CACHE["nc"]
    in_maps = host_inputs(x, W_qvk, b_qvk)
    from concourse.bass_utils import run_bass_kernel_spmd

    res = run_bass_kernel_spmd(
        nc,
        in_maps,
        core_ids=list(range(NCORES)),
        trace=bool(int(os.environ.get("KERNEL_TRACE", "0"))),
    )
    _CACHE["last_result"] = res
    out = np.concatenate(
        [np.asarray(r["out"], dtype=np.float32) for r in res.results], axis=1
    )
    return out.reshape(B, N, D)


# revision 40
# speedup vs baseline: 1.1177x; 1.0526x over previous
"""Multi-head causal linear attention (B=1, N=2048, D=1024, H=16) on 8 trn2 cores.

Math: reference computes, per head (e=64):
    q = softmax(q_raw, -1) * e**-0.5 ;  k = exp(k_raw)
    out_n = (q_n . KV_n) / (q_n . (kcum_n + EPS)),  KV_n = sum_{j<=n} k_j v_j^T
Because both numerator and denominator are linear in q_n, the softmax
normalization and the e**-0.5 scale cancel exactly; only u = exp(q_raw)
matters.  The EPS term contributes <1e-6 relative and is dropped.  The
v-bias contribution factors out:  out += b_v  (sum_j s_nj / denom ~= 1).

Per-core work (head-parallel, 2 heads/core):
    qvk^T = W_c^T @ x  computed as matmul(lhsT=W_block, rhs=x^T) on PE,
    x^T is pre-transposed on the host so no on-chip transpose is needed.
    Chunked causal linear attention (chunk=128) with the classic
    intra (masked QK^T V) + inter (running KV state) recurrence.

Layout tricks:
  - input DMA is issued first, in k-tile bundles [W_k | x0_k], and the
    first token tile's projection runs k-outer / f-inner so the PE starts
    real (HAM-warming) work as soon as bundle 0 lands.
  - the running KV state is kept block-diagonal [128, 2*65] in bf16 so a
    single matmul (lhsT=UT chunk, K=128 -> FWL) applies BOTH heads' inter
    term, and a single delta matmul (lhsT=ek_tok, rhs=v_aug flat) computes
    both heads' state update (off-diagonal blocks are garbage, never read).
  - per-chunk prepare (transpose/S/mask) is interleaved into the chain so
    the tail stays dense on PE (HAM stays at K=8/8).
"""

import os
from contextlib import ExitStack

import numpy as np

import concourse.bass as bass
import concourse.mybir as mybir
import concourse.tile as tile
from concourse import bacc
from concourse._compat import with_exitstack
from concourse.bass import ts

FP32 = mybir.dt.float32
BF16 = mybir.dt.bfloat16

B, N, D, H = 1, 2048, 1024, 16
E = D // H          # 64 head dim
NCORES = 8
HPC = H // NCORES   # 2 heads per core
F = 3 * HPC * E     # 384 per-core projected features (q | k | v)
KT = D // 128       # 8 contraction tiles
TT = 512            # token tile (projection granularity)
NTT = N // TT       # 4
C = 128             # chunk (tokens) for the causal recurrence
CPT = TT // C       # 4 chunks per token tile
NC = N // C         # 16 chunks total
BK = F + TT         # bundle cols per k-tile: [W(384) | x0(512)]
EA = E + 1          # 65: v columns + ones column

Exp = mybir.ActivationFunctionType.Exp
ActCopy = mybir.ActivationFunctionType.Copy
ActIdent = mybir.ActivationFunctionType.Identity
MULT = mybir.AluOpType.mult
ADD = mybir.AluOpType.add


@with_exitstack
def _emit(ctx: ExitStack, tc, io):
    nc = tc.nc
    bund_d, cf_d, xt1_d, xt23_d, out_d = io

    const = ctx.enter_context(tc.tile_pool(name="const", bufs=1))
    chain = ctx.enter_context(tc.tile_pool(name="chain", bufs=2))
    smtp = ctx.enter_context(tc.tile_pool(name="smtp", bufs=2))
    small = ctx.enter_context(tc.tile_pool(name="small", bufs=3))
    outp = ctx.enter_context(tc.tile_pool(name="outp", bufs=3))
    pproj = ctx.enter_context(tc.tile_pool(name="pproj", bufs=2, space="PSUM"))
    # per-head S^T banks (single-buffered, separate banks so the row-packed
    # matmul pair runs concurrently)
    ps_scr = ctx.enter_context(tc.tile_pool(name="ps_scr", bufs=1, space="PSUM"))
    # one bank, single-buffered: [tr(Ek) 128 | tr(V) 128] bf16
    ps_tr = ctx.enter_context(tc.tile_pool(name="ps_tr", bufs=1, space="PSUM"))
    # one 2KB bank per chunk: [out (130) | dd (130) | unused]; also hosts the
    # warm-up junk and tile-0's third projection accumulator (V)
    ps_od = ctx.enter_context(tc.tile_pool(name="ps_od", bufs=3, space="PSUM"))

    # ---- persistent SBUF ----
    # bund: [ (W_k(384) | x0_k(512)) * 8 | ident 128 ]
    bund_sb = const.tile([128, KT * BK + 128], BF16)
    cf_sb = const.tile([128, 3 + C], FP32)  # [bq|bk|bv|mask]
    xtr_sb = const.tile([128, (NTT - 1) * KT * TT], BF16)  # xt tt=1..3, (tt k t)
    kv_st = [
        const.tile([128, HPC * EA], BF16, name=f"kv{i}") for i in range(2)
    ]  # block-diag state

    id_sb = bund_sb[:, KT * BK : KT * BK + 128]
    bq_sb = cf_sb[:, 0:1]
    bk_sb = cf_sb[:, 1:2]
    bv_sb = cf_sb[:, 2:3]  # per-partition (= per V feature) bias column
    mask_sb = cf_sb[:, 3:]  # [128, 128]  (j, i) 1 iff j<=i

    def w_ap(k, f):
        return bund_sb[:, k * BK + f * 128 : k * BK + (f + 1) * 128]

    def xt_ap(tt, k):
        if tt == 0:
            return bund_sb[:, k * BK + F : k * BK + F + TT]
        base = (tt - 1) * KT * TT + k * TT
        return xtr_sb[:, base : base + TT]

    # ---- input DMA first, all on ONE ring (strictly ordered, full
    # bandwidth each) so early bundles are not starved by the xt stream.
    # cf (biases+mask) goes first — it is tiny and gates the tile-0 exps.
    # Per-k granularity keeps the tile-0 PE drip-fed without long stalls.
    nc.sync.dma_start(cf_sb[:, :], cf_d[:, :])
    for k in range(KT):
        hi = (k + 1) * BK if k < KT - 1 else KT * BK + 128
        nc.sync.dma_start(bund_sb[:, k * BK : hi], bund_d[:, k * BK : hi])
    nc.sync.dma_start(xtr_sb[:, 0 : KT * TT], xt1_d[:, :])
    nc.sync.dma_start(xtr_sb[:, KT * TT : 2 * KT * TT], xt23_d[:, 0 : KT * TT])
    nc.sync.dma_start(xtr_sb[:, 2 * KT * TT :], xt23_d[:, KT * TT :])

    # zero the off-diagonal blocks of both KV state buffers (they are only
    # ever written in their diagonal blocks)
    nc.gpsimd.memset(kv_st[0][:, :], 0.0)
    nc.gpsimd.memset(kv_st[1][:, :], 0.0)
    junk_sb = const.tile([128, 128], BF16, name="junk_sb")
    nc.gpsimd.memset(junk_sb[:, :], 0.0)

    # ---- HAM warm-up: ~2.6us of junk matmuls bridge the DMA-path startup
    # latency so the real tile-0 matmuls run at 2.4 GHz.  They occupy an od
    # bank that the chunk rotation reuses later.
    junk_ps = ps_od.tile([128, 512], FP32, tag="od", name="junkps")
    for _ in range(24):
        nc.tensor.matmul(
            junk_ps[:, 0:128],
            lhsT=junk_sb[:, :],
            rhs=junk_sb[:, :],
            start=True,
            stop=True,
        )

    st = [dict(smt=[None] * CPT, ek=[None] * CPT, va=[None] * CPT) for _ in range(NTT)]
    dma_flip = [0]

    def emit_act(tt, f, pp):
        s = st[tt]
        if f == 0:
            s["UT"] = UT = chain.tile([128, TT], BF16, tag="UT", name=f"UT{tt}")
            nc.scalar.activation(UT[:, :], pp[:, :], Exp, bias=bq_sb[:, 0:1])
        elif f == 1:
            s["EkT"] = EkT = chain.tile([128, TT], BF16, tag="EkT", name=f"EkT{tt}")
            nc.scalar.activation(EkT[:, :], pp[:, :], Exp, bias=bk_sb[:, 0:1])
        else:
            # fold the v-bias into V here: sum_j w_ij (v_j + bv) =
            # num_ij + den_i * bv, so out = num'/den needs no bias add.
            s["VT"] = VT = chain.tile([128, TT], BF16, tag="VT", name=f"VT{tt}")
            nc.scalar.activation(VT[:, :], pp[:, :], ActIdent, bias=bv_sb[:, 0:1])

    def emit_proj_f(tt, f):
        # projection (f-outer): qvk^T[f, t] = sum_d W[d, f] * xT[d, t]
        pp = pproj.tile([128, TT], FP32, tag="proj", name=f"pp{tt}_{f}")
        for k in range(KT):
            nc.tensor.matmul(
                pp[:, :],
                lhsT=w_ap(k, f),
                rhs=xt_ap(tt, k),
                start=(k == 0),
                stop=(k == KT - 1),
            )
        emit_act(tt, f, pp)

    def emit_proj_tile0():
        # k-outer / f-inner: each arriving bundle feeds 3 matmuls, PE ramps
        # with the DMA stream and warms HAM on real work.  The third
        # accumulator (V) borrows an od bank so pproj stays at 2 banks.
        pps = [
            pproj.tile([128, TT], FP32, tag="proj", name="pp0_0"),
            pproj.tile([128, TT], FP32, tag="proj", name="pp0_1"),
            ps_od.tile([128, TT], FP32, tag="od", name="pp0_2"),
        ]
        for k in range(KT):
            for f in range(3):
                nc.tensor.matmul(
                    pps[f][:, :],
                    lhsT=w_ap(k, f),
                    rhs=xt_ap(0, k),
                    start=(k == 0),
                    stop=(k == KT - 1),
                )
        for f in range(3):
            emit_act(0, f, pps[f])

    def emit_prep(tt, cc):
        # per-chunk: token-layout Ek / V (PE transpose + ACT copy), chunk
        # scores S^T and mask
        s = st[tt]
        UT, EkT, VT = s["UT"], s["EkT"], s["VT"]
        # S^T[j, i] = sum_d Ek[j,d] U[i,d]  (row-packed head pair, separate
        # PSUM banks so the two matmuls overlap in the array)
        smts = []
        sps_t = []
        for h in range(HPC):
            sps = ps_scr.tile([128, C], FP32, tag=f"s{h}", name=f"sp{tt}_{cc}_{h}")
            nc.tensor.matmul(
                sps[:, :],
                lhsT=EkT[ts(h, E), ts(cc, C)],
                rhs=UT[ts(h, E), ts(cc, C)],
                start=True,
                stop=True,
                tile_position=(E * h, 0),
            )
            sps_t.append(sps)
        trp = ps_tr.tile([128, 256], BF16, tag="tr", name=f"trp{tt}_{cc}")
        nc.tensor.transpose(trp[:, 0:128], EkT[:, ts(cc, C)], id_sb[:, :])
        nc.tensor.transpose(trp[:, 128:256], VT[:, ts(cc, C)], id_sb[:, :])
        for h in range(HPC):
            sm = smtp.tile([128, C], BF16, tag=f"m{h}", bufs=5, name=f"smt{tt}_{cc}_{h}")
            nc.vector.tensor_mul(sm[:, :], sps_t[h][:, :], mask_sb[:, :])
            smts.append(sm)
        ek_tok = small.tile([128, 128], BF16, tag="ektok", bufs=6, name=f"ek{tt}_{cc}")
        nc.scalar.copy(ek_tok[:, :], trp[:, 0:128])
        v_aug = small.tile(
            [128, HPC, EA], BF16, tag="vaug", bufs=6, name=f"va{tt}_{cc}"
        )
        nc.scalar.copy(
            v_aug[:, :, 0:E],
            trp[:, 128:256].rearrange("p (g e) -> p g e", g=HPC),
        )
        nc.gpsimd.memset(v_aug[:, :, E : E + 1], 1.0)
        s["smt"][cc] = smts
        s["ek"][cc] = ek_tok
        s["va"][cc] = v_aug

    osb_ref = [None]

    def finalize(out_ps, tt, cc):
        c = tt * CPT + cc
        last_chunk = c == NC - 1
        if cc == 0:
            osb_ref[0] = outp.tile([128, CPT, HPC * E], BF16, tag="osb", name=f"o{tt}")
        osb = osb_ref[0]
        o3 = out_ps.rearrange("p (g e) -> p g e", g=HPC)
        rec = small.tile([128, HPC], FP32, tag="rec", name=f"rec{c}")
        nc.vector.reciprocal(rec[:, :], o3[:, :, E])
        for h in range(HPC):
            # v-bias is folded into V, so normalize is a pure per-token scale
            nc.vector.tensor_scalar_mul(
                osb[:, cc, ts(h, E)], o3[:, h, 0:E], rec[:, h : h + 1]
            )
            if last_chunk:
                eng2 = nc.sync if h == 0 else nc.gpsimd
                eng2.dma_start(out_d[ts(c, C), ts(h, E)], osb[:, cc, ts(h, E)])
        if last_chunk:
            return
        eng = nc.gpsimd if dma_flip[0] % 2 else nc.sync
        if tt == NTT - 1:
            dma_flip[0] += 1
            eng.dma_start(out_d[ts(c, C), :], osb[:, cc, :])
        elif cc == CPT - 1:
            dma_flip[0] += 1
            eng.dma_start(
                out_d[ts(tt, TT), :].rearrange("(cc p) f -> p cc f", p=128),
                osb[:, :, :],
            )

    def emit_chain_chunk(tt, cc):
        s = st[tt]
        UT = s["UT"]
        c = tt * CPT + cc
        smts, ek_tok, v_aug = s["smt"][cc], s["ek"][cc], s["va"][cc]
        vflat = v_aug.rearrange("p g e -> p (g e)")
        od = ps_od.tile([128, 512], FP32, tag="od", name=f"od{c}")
        out_ps = od[:, 0 : HPC * EA]
        # NOTE on start=: start=True clears has_written for the WHOLE bank,
        # so only the FIRST matmul touching this od bank per chunk may set
        # it; later matmuls rely on per-element overwrite-where-unwritten.
        if c < NC - 1:
            # both heads' state delta in one matmul; off-diagonal blocks of
            # dd are cross-head garbage and never read.  Emitted first so
            # the DVE state update has slack before the next chunk's inter.
            dd = od[:, HPC * EA : 2 * HPC * EA]
            nc.tensor.matmul(
                dd[:, :],
                lhsT=ek_tok[:, :],
                rhs=vflat[:, :],
                start=True,
                stop=True,
            )
            nxt = kv_st[c % 2]
            if c == 0:
                nc.vector.tensor_copy(nxt[0:E, 0:EA], dd[0:E, 0:EA])
                nc.vector.tensor_copy(nxt[E:128, EA:], dd[E:128, EA:])
            else:
                prv = kv_st[(c - 1) % 2]
                nc.vector.tensor_add(nxt[0:E, 0:EA], dd[0:E, 0:EA], prv[0:E, 0:EA])
                nc.vector.tensor_add(nxt[E:128, EA:], dd[E:128, EA:], prv[E:128, EA:])
        if tt == NTT - 1:
            # hold the PE clock warm while the DVE state update runs; junk
            # lands in this chunk's od bank's unused columns (start=False so
            # the live accumulation's has_written bits survive)
            emit_filler(5, target=od[0:64, 280:344], start=False)
        for h in range(HPC):
            nc.tensor.matmul(
                out_ps[:, h * EA : (h + 1) * EA],
                lhsT=smts[h][:, :],
                rhs=v_aug[:, h, :],
                start=(c == NC - 1 and h == 0),
                stop=(c == 0 and h == HPC - 1),
            )
        if c > 0:
            # both heads' inter term in one K=128 matmul against the
            # block-diagonal bf16 state
            nc.tensor.matmul(
                out_ps[:, :],
                lhsT=UT[:, ts(cc, C)],
                rhs=kv_st[(c - 1) % 2][:, :],
                start=False,
                stop=True,
            )
        finalize(out_ps, tt, cc)

    def emit_filler(n, target=None, start=True):
        # tiny junk matmuls keep the PE activity monitor from re-throttling
        # across short dependency stalls.  With start=False they may share a
        # live bank's unused columns without clearing its has_written bits.
        if target is None:
            jp = ps_scr.tile([128, C], FP32, tag="s0", name="fill")
            target = jp[0:64, 0:64]
        for _ in range(n):
            nc.tensor.matmul(
                target,
                lhsT=junk_sb[:, 0:64],
                rhs=junk_sb[:, 0:64],
                start=start,
                stop=True,
            )

    # ---- emission: tile-0 projection (k-outer); tile-1's f0 projection goes
    # right behind it so the PE has work while the scalar engine runs the
    # tile-0 exps.  Then per tile: chain chunks with next-chunk prepare and
    # next-tile projection woven in between.
    emit_proj_tile0()
    emit_proj_f(1, 0)
    # bridge the exp/copy latency at the tile-0 -> tile-1 boundary so the
    # activity monitor never sees an idle window there
    emit_filler(16)
    emit_prep(0, 0)
    for tt in range(NTT):
        if tt == 0:
            slices = [lambda: emit_proj_f(1, 1), lambda: emit_proj_f(1, 2), None]
        elif tt < NTT - 1:
            slices = [lambda f=f: emit_proj_f(tt + 1, f) for f in range(3)]
        else:
            slices = [None] * 3
        for cc in range(CPT):
            emit_chain_chunk(tt, cc)
            if cc < CPT - 1:
                emit_prep(tt, cc + 1)
                if slices[cc] is not None:
                    slices[cc]()
        if tt < NTT - 1:
            emit_prep(tt + 1, 0)


def build_nc():
    nc = bacc.Bacc(
        "TRN2",
        target_bir_lowering=False,
        debug=False,
        enable_asserts=False,
        num_devices=NCORES,
    )
    bund_d = nc.dram_tensor(
        "bund", [128, KT * BK + 128], BF16, kind="ExternalInput"
    ).ap()
    cf_d = nc.dram_tensor("cf", [128, 3 + C], FP32, kind="ExternalInput").ap()
    xt1_d = nc.dram_tensor("xt1", [128, KT * TT], BF16, kind="ExternalInput").ap()
    xt23_d = nc.dram_tensor(
        "xt23", [128, 2 * KT * TT], BF16, kind="ExternalInput"
    ).ap()
    out_d = nc.dram_tensor("out", [N, HPC * E], BF16, kind="ExternalOutput").ap()
    io = (bund_d, cf_d, xt1_d, xt23_d, out_d)
    with tile.TileContext(nc) as tc:
        _emit(tc, io)
    nc.compile()
    return nc


def host_inputs(x, W_qvk, b_qvk):
    """Full inputs -> per-core in_maps (host-side shard + transpose)."""
    import ml_dtypes

    x = np.asarray(x, dtype=np.float32).reshape(N, D)
    W = np.asarray(W_qvk, dtype=np.float32)
    b = np.asarray(b_qvk, dtype=np.float32)
    xt = x.T.astype(ml_dtypes.bfloat16)  # (D, N)

    def pack(a):  # (D, M) -> (128, KT*M), partition-contiguous
        kt, m = a.shape[0] // 128, a.shape[1]
        return np.ascontiguousarray(
            a.reshape(kt, 128, m).transpose(1, 0, 2).reshape(128, kt * m)
        )

    xtp = [pack(xt[:, tt * TT : (tt + 1) * TT]) for tt in range(NTT)]
    xt1 = xtp[1]
    xt23 = np.ascontiguousarray(np.concatenate([xtp[2], xtp[3]], axis=1))
    ident = np.eye(128, dtype=ml_dtypes.bfloat16)

    tri = np.tril(np.ones((C, C), dtype=np.float32))  # [i, j] valid j<=i
    mask = np.ascontiguousarray(tri.T)  # [j, i] 1 iff j<=i

    in_maps = []
    for core in range(NCORES):
        heads = [HPC * core + i for i in range(HPC)]
        # torch.chunk order in reference: q, v, k
        qcols = np.concatenate([np.arange(E * h, E * h + E) for h in heads])
        vcols = qcols + D
        kcols = qcols + 2 * D
        Wc = pack(
            np.concatenate([W[:, qcols], W[:, kcols], W[:, vcols]], axis=1).astype(
                ml_dtypes.bfloat16
            )
        )
        # bundle layout per k: [W_k (384) | x0_k (512)], then ident
        bund = np.empty((128, KT * BK + 128), dtype=ml_dtypes.bfloat16)
        for k in range(KT):
            bund[:, k * BK : k * BK + F] = Wc[:, k * F : (k + 1) * F]
            bund[:, k * BK + F : (k + 1) * BK] = xtp[0][:, k * TT : (k + 1) * TT]
        bund[:, KT * BK :] = ident
        bq = b[qcols].reshape(128, 1)
        bk = b[kcols].reshape(128, 1)
        bv = b[vcols].reshape(128, 1)
        cf = np.ascontiguousarray(
            np.concatenate([bq, bk, bv, mask], axis=1, dtype=np.float32)
        )
        in_maps.append(dict(bund=bund, cf=cf, xt1=xt1, xt23=xt23))
    return in_maps


_CACHE = {}


def kernel(x, W_qvk, b_qvk, head_num):
    assert int(np.asarray(head_num)) == H
    if "nc" not in _CACHE:
        _CACHE["nc"] = build_nc()
    nc = _CACHE["nc"]
    in_maps = host_inputs(x, W_qvk, b_qvk)
    from concourse.bass_utils import run_bass_kernel_spmd

    res = run_bass_kernel_spmd(
        nc,
        in_maps,
        core_ids=list(range(NCORES)),
        trace=bool(int(os.environ.get("KERNEL_TRACE", "0"))),
    )
    _CACHE["last_result"] = res
    out = np.concatenate(
        [np.asarray(r["out"], dtype=np.float32) for r in res.results], axis=1
    )
    return out.reshape(B, N, D)
